# revision 18
# baseline (speedup 1.0000x reference)
"""AttentionLSTMDecoder — hand-written Bass/Tile kernel, 8-core data-parallel.

Sharding: batch B=16 -> 2 per NeuronCore (recurrence is batch-independent,
no collectives).  The whole 128-step recurrence + attention + output
projection runs as ONE Bass program per core.

Math restructuring (vs the nn.Module):
  * zero-state LSTM cell: forget gate is dead; only i/g/o rows of W_ih used.
  * c-zoneout chain is dead (h_new uses c_new) -> dropped.
  * x_t@W_ih part of gates hoisted into one PRE GEMM over all 128 steps.
  * sigmoid(x) = 0.5 + 0.5*tanh(x/2): the whole loop needs only {tanh, exp}
    which live in ONE ScalarE table set (no 2.7us table reloads per step).
  * per-step energy field X[a,t] = enc_ctx + s_t + wfb*accum built IN PSUM:
    rank-1 matmul (wfb x accum) + identity-matmul (enc_ctx) accumulate, then
    ONE ScalarE pass computes tanh(psum + s_bias) (s enters as the per-
    partition activation bias) -> VectorE untouched by the big tensor.
  * e = v . tanh(X) via matmul with v as 1-column stationary (a on partitions).
  * softmax without max-pass (|e| <= sum|v| ~ 26, exp stays in f32 range;
    masked lanes get -1e4 -> exp underflows to exact 0).  exp's free-dim
    accumulator gives the denominator for free.
  * maxout pairs are made partition-aligned by permuting W_readout rows
    (evens||odds) on the host, so MaxOut(2) is a plain tensor_tensor max.

Layouts (per core, B_loc=2):  a-major energies [a_part, t_free]; gates/h in
[h%128, (htile, b)]; s in [a%128, (atile, b)]; ctx in [d%128, (dtile, b)].
"""

import numpy as np
import ml_dtypes

import concourse.bass as bass
import concourse.bacc as bacc
import concourse.tile as tile
import concourse.mybir as mybir
from concourse.bass_utils import run_bass_kernel_spmd
from concourse.masks import make_identity

BF = mybir.dt.bfloat16
F32 = mybir.dt.float32
FP8 = mybir.dt.float8e4
NPBF = ml_dtypes.bfloat16
NPF8 = ml_dtypes.float8_e4m3
AF = mybir.ActivationFunctionType
ALU = mybir.AluOpType

NDEV = 8
ZH = 0.05

# full-size problem config
CFG = dict(Bl=2, T=500, D=512, A=1024, H=1024, E=640, N=128, V=10025)


def _d(cfg):
    Bl, T, D, A, H, E, N, V = (cfg[k] for k in ("Bl", "T", "D", "A", "H", "E", "N", "V"))
    G3 = 3 * H
    return dict(
        Bl=Bl, T=T, D=D, A=A, H=H, E=E, N=N, V=V, G3=G3,
        TQ=T // 4, DT=D // 128, AT=A // 128, HT=H // 128, ET=E // 128,
        GT=G3 // 128, RO=H + E + D, CTRO=(H + E + D) // 128, PTRO=H // 128,
        P2=H // 2, PT2=H // 256, NB=N * Bl, VT=(V + 127) // 128,
    )


def build(nc, cfg):
    c = _d(cfg)
    Bl, T, TQ, D, DT, A, AT, H, HT, E, ET, N, V, VT = (
        c[k] for k in ("Bl", "T", "TQ", "D", "DT", "A", "AT", "H", "HT", "E", "ET", "N", "V", "VT"))
    G3, GT, CTRO, PTRO, PT2, NB = (c[k] for k in ("G3", "GT", "CTRO", "PTRO", "PT2", "NB"))

    # ---------------- DRAM I/O ----------------
    def din(name, shape, dt=BF):
        return nc.dram_tensor(name, shape, dt, kind="ExternalInput")

    encT_d = din("encT", [Bl, D, T])
    enc_t_d = din("enc_t", [Bl, 4, TQ, D])
    embT_d = din("embT", [ET, 128, NB])
    WencT_d = din("WencT", [DT, 128, A])
    benc_d = din("benc", [128, AT], F32)
    wif_d = din("wif", [128, DT])
    WxT_d = din("WxT", [ET, 128, G3])
    bigo_d = din("bigo", [128, GT], F32)
    WcT_d = din("WcT", [DT, 128, G3], FP8)
    WsT_d = din("WsT", [HT, 128, A], FP8)
    wfb_d = din("wfb", [1, A])
    vatt_d = din("vatt", [128, AT])
    mask_d = din("maskadd", [Bl, T], F32)
    WroT_d = din("WroT", [CTRO, 128, H])
    bro_d = din("bro", [128, PTRO], F32)
    WoT_d = din("WoT", [PT2, 128, V])
    bout_d = din("bout", [128, VT], F32)
    out_d = nc.dram_tensor("out_d", [V, NB], F32, kind="ExternalOutput")

    with tile.TileContext(nc) as tc:
        with (
            tc.tile_pool(name="const", bufs=1) as cp,
            tc.tile_pool(name="state", bufs=2) as sp,
            tc.tile_pool(name="bigstep", bufs=3) as bsp,
            tc.tile_pool(name="smallstep", bufs=3) as ssp,
            tc.tile_pool(name="rowstep", bufs=1) as rsp,
        ):
            # persistent SBUF tensors
            ident = cp.tile([128, 128], BF, tag="ident")
            make_identity(nc, ident[:])
            one32 = cp.tile([1, 1], F32, tag="one32")
            nc.vector.memset(one32[:], 1.0)

            encT = cp.tile([128, Bl, DT, T], BF, tag="encT")
            enc_t = cp.tile([128, Bl, 4, D], BF, tag="enc_t")
            embT = cp.tile([128, ET, NB], BF, tag="embT")
            WcT = cp.tile([128, DT, G3], FP8, tag="WcT")
            WsT = cp.tile([128, HT, A], FP8, tag="WsT")
            wfb = cp.tile([1, A], BF, tag="wfb")
            vatt = cp.tile([128, AT], BF, tag="vatt")
            mask_b = [cp.tile([1, T], F32, tag=f"mask{b}", name=f"mask{b}") for b in range(Bl)]
            ivf_b = [cp.tile([1, T], F32, tag=f"ivf{b}", name=f"ivf{b}") for b in range(Bl)]
            enc_ctx = cp.tile([128, Bl, AT, T], BF, tag="enc_ctx")
            gx = cp.tile([128, GT, N, Bl], BF, tag="gx")
            h_seq = cp.tile([128, HT, N, Bl], BF, tag="h_seq")
            ctx_seq = cp.tile([128, DT, N, Bl], BF, tag="ctx_seq")

            for b in range(Bl):
                for dt in range(DT):
                    nc.sync.dma_start(encT[:, b, dt, :], encT_d[b, dt * 128:(dt + 1) * 128, :])
                for tt in range(4):
                    nc.sync.dma_start(enc_t[0:TQ, b, tt, :], enc_t_d[b, tt, :, :])
                nc.sync.dma_start(mask_b[b][:], mask_d[b:b + 1, :])
            for et in range(ET):
                nc.sync.dma_start(embT[:, et, :], embT_d[et])
            for dt in range(DT):
                nc.sync.dma_start(WcT[:, dt, :], WcT_d[dt])
            for ht in range(HT):
                nc.sync.dma_start(WsT[:, ht, :], WsT_d[ht])
            nc.sync.dma_start(wfb[:], wfb_d[:])
            nc.sync.dma_start(vatt[:], vatt_d[:])

            # ---------------- PRE phase ----------------
            with (
                tc.tile_pool(name="prew", bufs=1) as pw,
                tc.tile_pool(name="ps_ec", bufs=2, space="PSUM") as ps_ec,
                tc.tile_pool(name="ps_if", bufs=1, space="PSUM") as ps_if,
                tc.tile_pool(name="presb", bufs=1) as psb,
            ):
                WencT = pw.tile([128, DT, A], BF, tag="WencT")
                wif = pw.tile([128, DT], BF, tag="wif")
                benc = pw.tile([128, AT], F32, tag="benc")
                for dt in range(DT):
                    nc.sync.dma_start(WencT[:, dt, :], WencT_d[dt])
                nc.sync.dma_start(wif[:], wif_d[:])
                nc.sync.dma_start(benc[:], benc_d[:])

                # enc_ctx[b, at] = W_enc_ctx @ enc_b^T + b_enc_ctx   (a-major)
                for b in range(Bl):
                    for at in range(AT):
                        ecp = ps_ec.tile([128, T], F32, tag="ecp")
                        for dt in range(DT):
                            nc.tensor.matmul(
                                ecp[:], WencT[:, dt, at * 128:(at + 1) * 128],
                                encT[:, b, dt, :], start=(dt == 0), stop=(dt == DT - 1))
                        nc.scalar.activation(enc_ctx[:, b, at, :], ecp[:],
                                             AF.Identity, bias=benc[:, at:at + 1])

                # inv_fert_half = 0.25*tanh(x/2)+0.25, x = enc @ w_if
                for b in range(Bl):
                    ifp = ps_if.tile([1, T], F32, tag="ifp")
                    for dt in range(DT):
                        nc.tensor.matmul(ifp[:], wif[:, dt:dt + 1], encT[:, b, dt, :],
                                         start=(dt == 0), stop=(dt == DT - 1))
                    th0 = psb.tile([1, T], F32, tag="ifth")
                    nc.scalar.activation(th0[:], ifp[:], AF.Tanh, scale=0.5)
                    nc.vector.tensor_scalar(ivf_b[b][:], th0[:], 0.25, 0.25,
                                            op0=ALU.mult, op1=ALU.add)

            # gx[ht, n, b] = emb_shift @ Wx^T + b_igo
            with (
                tc.tile_pool(name="prew2", bufs=1) as pw2,
                tc.tile_pool(name="ps_gx", bufs=2, space="PSUM") as ps_gx,
            ):
                WxT = pw2.tile([128, ET, G3], BF, tag="WxT")
                bigo = pw2.tile([128, GT], F32, tag="bigo")
                for et in range(ET):
                    nc.sync.dma_start(WxT[:, et, :], WxT_d[et])
                nc.sync.dma_start(bigo[:], bigo_d[:])
                for ht in range(GT):
                    gxp = ps_gx.tile([128, NB], F32, tag="gxp")
                    for et in range(ET):
                        nc.tensor.matmul(gxp[:], WxT[:, et, ht * 128:(ht + 1) * 128],
                                         embT[:, et, :], start=(et == 0), stop=(et == ET - 1))
                    nc.scalar.activation(gx[:, ht, :, :], gxp[:],
                                         AF.Identity, bias=bigo[:, ht:ht + 1])

            # ---------------- recurrence ----------------
            with (
                tc.tile_pool(name="ps_g", bufs=1, space="PSUM") as ps_g,
                tc.tile_pool(name="ps_s", bufs=1, space="PSUM") as ps_s,
                tc.tile_pool(name="ps_x", bufs=2, space="PSUM") as ps_x,
                tc.tile_pool(name="ps_e", bufs=2, space="PSUM") as ps_e,
                tc.tile_pool(name="ps_t", bufs=1, space="PSUM") as ps_t,
                tc.tile_pool(name="ps_c", bufs=1, space="PSUM") as ps_c,
            ):
                # PE warm-up: ~5us of back-to-back matmuls so the HAM clock
                # gate un-throttles to 2.4 GHz right at the recurrence head.
                # Reading gx (the last PRE output) pins these after PRE in
                # the schedule so the burst fills the PRE->loop gap.
                for wu in range(22):
                    wup = ps_x.tile([128, T], F32, tag="xp", name="wup")
                    nc.tensor.matmul(wup[:, 0:NB], ident[:],
                                     gx[:, wu % GT, :, :], start=True, stop=True)

                h_prev = None
                ctx_sb = None
                accum = [None] * Bl
                accum_bf = [None] * Bl
                for t in range(N):
                    # ---- gates = gx[t] + ctx @ Wc^T ----
                    gates = ssp.tile([128, GT, Bl], BF, tag="gates")
                    if t == 0:
                        nc.vector.tensor_copy(gates[:], gx[:, :, 0, :])
                    else:
                        gp = ps_g.tile([128, GT, Bl], F32, tag="gp")
                        for ht in range(GT):
                            for dt in range(DT):
                                nc.tensor.matmul(
                                    gp[:, ht, :], WcT[:, dt, ht * 128:(ht + 1) * 128],
                                    ctx_sb[:, dt, :], start=(dt == 0), stop=(dt == DT - 1))
                        nc.vector.tensor_add(gates[:], gp[:], gx[:, :, t, :])

                    # ---- LSTM cell nonlinearities (tanh-only forms) ----
                    ti = ssp.tile([128, HT, Bl], BF, tag="ti")
                    tg = ssp.tile([128, HT, Bl], BF, tag="tg")
                    to = ssp.tile([128, HT, Bl], BF, tag="to")
                    nc.scalar.activation(ti[:], gates[:, 0:HT, :], AF.Tanh, scale=0.5)
                    nc.scalar.activation(tg[:], gates[:, HT:2 * HT, :], AF.Tanh)
                    nc.scalar.activation(to[:], gates[:, 2 * HT:3 * HT, :], AF.Tanh, scale=0.5)
                    u = ssp.tile([128, HT, Bl], BF, tag="u")
                    nc.vector.scalar_tensor_tensor(u[:], ti[:], 1.0, tg[:],
                                                   op0=ALU.add, op1=ALU.mult)
                    tc_ = ssp.tile([128, HT, Bl], BF, tag="tc_")
                    nc.scalar.activation(tc_[:], u[:], AF.Tanh, scale=0.5)
                    v2 = ssp.tile([128, HT, Bl], BF, tag="v2")
                    nc.vector.scalar_tensor_tensor(v2[:], to[:], 1.0, tc_[:],
                                                   op0=ALU.add, op1=ALU.mult)
                    h = sp.tile([128, HT, Bl], BF, tag="h")
                    if t == 0:
                        nc.vector.tensor_scalar_mul(h[:], v2[:], (1.0 - ZH) / 2.0)
                    else:
                        t1 = ssp.tile([128, HT, Bl], BF, tag="t1")
                        nc.vector.tensor_scalar_mul(t1[:], v2[:], (1.0 - ZH) / 2.0)
                        nc.vector.scalar_tensor_tensor(h[:], h_prev[:], ZH, t1[:],
                                                       op0=ALU.mult, op1=ALU.add)
                    nc.vector.tensor_copy(h_seq[:, :, t, :], h[:])
                    h_prev = h

                    # ---- s = h @ Ws^T  (a-major out) ----
                    sps = ps_s.tile([128, AT, Bl], F32, tag="sps")
                    for at in range(AT):
                        for ht in range(HT):
                            nc.tensor.matmul(
                                sps[:, at, :], WsT[:, ht, at * 128:(at + 1) * 128],
                                h[:, ht, :], start=(ht == 0), stop=(ht == HT - 1))
                    s_sb = ssp.tile([128, AT, Bl], F32, tag="s_sb")
                    nc.vector.tensor_copy(s_sb[:], sps[:])

                    # ---- energies + softmax per b ----
                    ewn = []
                    for b in range(Bl):
                        eps = ps_e.tile([1, T], F32, tag="eps")
                        for at in range(AT):
                            xp = ps_x.tile([128, T], F32, tag="xp")
                            nc.tensor.matmul(xp[:], ident[:], enc_ctx[:, b, at, :],
                                             start=True, stop=(t == 0))
                            if t > 0:
                                nc.tensor.matmul(xp[:], wfb[0:1, at * 128:(at + 1) * 128],
                                                 accum_bf[b][:], start=False, stop=True)
                            th = bsp.tile([128, T], BF, tag="th")
                            nc.scalar.activation(th[:], xp[:], AF.Tanh,
                                                 bias=s_sb[:, at, b:b + 1])
                            nc.tensor.matmul(eps[:], vatt[:, at:at + 1], th[:],
                                             start=(at == 0), stop=(at == AT - 1))
                        em = rsp.tile([1, T], F32, tag=f"em{b}")
                        ew = rsp.tile([1, T], F32, tag=f"ew{b}")
                        ewn_t = rsp.tile([1, T], F32, tag=f"ewn{b}")
                        Ssum = ssp.tile([1, 1], F32, tag=f"Ssum{b}")
                        Sr = ssp.tile([1, 1], F32, tag=f"Sr{b}")
                        nc.vector.tensor_add(em[:], eps[:], mask_b[b][:])
                        nc.scalar.activation(ew[:], em[:], AF.Exp, accum_out=Ssum[:])
                        nc.vector.reciprocal(Sr[:], Ssum[:])
                        nc.vector.tensor_scalar_mul(ewn_t[:], ew[:], Sr[:])
                        ewn.append(ewn_t)

                        # accum += w * inv_fert/2
                        acc_new = sp.tile([1, T], F32, tag=f"accum{b}", name=f"accum{b}")
                        if t == 0:
                            nc.vector.tensor_mul(acc_new[:], ewn_t[:], ivf_b[b][:])
                        else:
                            wd = rsp.tile([1, T], F32, tag=f"wd{b}")
                            nc.vector.tensor_mul(wd[:], ewn_t[:], ivf_b[b][:])
                            nc.vector.tensor_add(acc_new[:], accum[b][:], wd[:])
                        accum[b] = acc_new
                        abf = sp.tile([1, T], BF, tag=f"accum_bf{b}", name=f"abf{b}", bufs=1)
                        nc.vector.tensor_copy(abf[:], acc_new[:])
                        accum_bf[b] = abf

                    # ---- ctx = w @ enc  (d-major out), via ew^T ----
                    ewT = ps_t.tile([128, 4, Bl], F32, tag="ewT")
                    for b in range(Bl):
                        for tt in range(4):
                            nc.tensor.matmul(ewT[0:TQ, tt, b:b + 1],
                                             ewn[b][0:1, tt * TQ:(tt + 1) * TQ],
                                             one32[:], start=True, stop=True)
                    ewT_sb = ssp.tile([128, 4, Bl], BF, tag="ewT_sb")
                    nc.vector.tensor_copy(ewT_sb[0:TQ, :, :], ewT[0:TQ, :, :])
                    cps = ps_c.tile([128, DT, Bl], F32, tag="cps")
                    for b in range(Bl):
                        for dt in range(DT):
                            for tt in range(4):
                                nc.tensor.matmul(
                                    cps[:, dt, b:b + 1],
                                    enc_t[0:TQ, b, tt, dt * 128:(dt + 1) * 128],
                                    ewT_sb[0:TQ, tt, b:b + 1],
                                    start=(tt == 0), stop=(tt == 3))
                    ctx_sb = sp.tile([128, DT, Bl], BF, tag="ctx_sb")
                    nc.vector.tensor_copy(ctx_sb[:], cps[:])
                    nc.scalar.copy(ctx_seq[:, :, t, :], cps[:])

            # ---------------- POST phase ----------------
            with (
                tc.tile_pool(name="postw", bufs=1) as pow_,
                tc.tile_pool(name="postwo", bufs=3) as powo,
                tc.tile_pool(name="post_sb", bufs=4) as post_sb,
            ):
                bro = pow_.tile([128, PTRO], F32, tag="bro")
                bout = pow_.tile([128, VT], F32, tag="bout")
                mo = pow_.tile([128, PT2, NB], BF, tag="mo")
                half = pow_.tile([128, NB], F32, tag="half")
                nc.sync.dma_start(bro[:], bro_d[:])
                nc.sync.dma_start(bout[:], bout_d[:])

                def ro_rhs(ct):
                    if ct < HT:
                        return h_seq[:, ct, :, :]
                    if ct < HT + ET:
                        return embT[:, ct - HT, :]
                    return ctx_seq[:, ct - HT - ET, :, :]

                # readout: all PTRO output tiles accumulate in one psum tensor,
                # each pt slice padded to a full bank (512 f32)
                with (
                    tc.tile_pool(name="postro", bufs=2) as powr,
                    tc.tile_pool(name="ps_rp", bufs=1, space="PSUM") as ps_rp,
                ):
                    rp = ps_rp.tile([128, PTRO, 512], F32, tag="rp")
                    for ct in range(CTRO):
                        WroTt = powr.tile([128, H], BF, tag="WroTt")
                        nc.sync.dma_start(WroTt[:], WroT_d[ct])
                        for pt in range(PTRO):
                            nc.tensor.matmul(rp[:, pt, 0:NB],
                                             WroTt[:, pt * 128:(pt + 1) * 128],
                                             ro_rhs(ct), start=(ct == 0),
                                             stop=(ct == CTRO - 1))
                    # maxout(2) with per-half biases, pairs partition-aligned
                    # by the host-side W_readout row permutation
                    for k in range(PT2):
                        nc.vector.tensor_scalar_add(half[:], rp[:, k + PT2, 0:NB],
                                                    bro[:, k + PT2:k + PT2 + 1])
                        nc.vector.scalar_tensor_tensor(mo[:, k, :], rp[:, k, 0:NB],
                                                       bro[:, k:k + 1], half[:],
                                                       op0=ALU.add, op1=ALU.max)

                with tc.tile_pool(name="ps_lp", bufs=3, space="PSUM") as ps_lp:
                    for vt in range(VT):
                        vp = min(128, V - vt * 128)
                        wo = powo.tile([128, PT2, 128], BF, tag="wo")
                        for ct in range(PT2):
                            nc.sync.dma_start(wo[:, ct, 0:vp],
                                              WoT_d[ct, :, vt * 128:vt * 128 + vp])
                        lp = ps_lp.tile([128, NB], F32, tag="lp")
                        for ct in range(PT2):
                            nc.tensor.matmul(lp[0:vp, :], wo[:, ct, 0:vp], mo[:, ct, :],
                                             start=(ct == 0), stop=(ct == PT2 - 1))
                        lg = post_sb.tile([128, NB], F32, tag="lg")
                        if vt % 2 == 0:
                            nc.vector.tensor_scalar_add(lg[0:vp, :], lp[0:vp, :],
                                                        bout[0:vp, vt:vt + 1])
                        else:
                            nc.scalar.activation(lg[0:vp, :], lp[0:vp, :], AF.Identity,
                                                 bias=bout[0:vp, vt:vt + 1])
                        nc.sync.dma_start(out_d[vt * 128:vt * 128 + vp, :], lg[0:vp, :])
    return nc


# ---------------------------------------------------------------------------
# host side
# ---------------------------------------------------------------------------

def prep_weights(cfg, embed, W_ih, b_ih, b_hh, W_s, W_enc_ctx, b_enc_ctx, v_att,
                 W_inv_fert, W_fb, W_readout, b_readout, W_out, b_out):
    """Static (input-independent) weight layout transforms -> shared arrays."""
    c = _d(cfg)
    H, E, A, D, V = c["H"], c["E"], c["A"], c["D"], c["V"]
    rows = np.r_[0:H, 2 * H:4 * H]
    Wih = np.asarray(W_ih, np.float32)
    Wx = Wih[rows, :E]
    Wc = Wih[rows, E:]
    b_igo = (np.asarray(b_ih, np.float32) + np.asarray(b_hh, np.float32))[rows]
    Wro = np.asarray(W_readout, np.float32)
    bro_f = np.asarray(b_readout, np.float32)
    Wro_p = np.concatenate([Wro[0::2], Wro[1::2]], axis=0)
    bro_p = np.concatenate([bro_f[0::2], bro_f[1::2]], axis=0)

    def bf(x):
        return np.ascontiguousarray(np.asarray(x, np.float32)).astype(NPBF)

    w = {
        "WxT": bf(Wx.T).reshape(c["ET"], 128, c["G3"]),
        "WcT": np.ascontiguousarray(Wc.T).astype(NPF8).reshape(c["DT"], 128, c["G3"]),
        "WsT": np.ascontiguousarray(np.asarray(W_s, np.float32).T).astype(NPF8).reshape(c["HT"], 128, A),
        "WencT": bf(np.asarray(W_enc_ctx, np.float32).T).reshape(c["DT"], 128, A),
        "benc": np.ascontiguousarray(
            np.asarray(b_enc_ctx, np.float32).reshape(c["AT"], 128).T),
        "bigo": np.ascontiguousarray(b_igo.reshape(c["GT"], 128).T),
        "vatt": bf(np.asarray(v_att, np.float32)[0].reshape(c["AT"], 128).T),
        "wfb": bf(np.asarray(W_fb, np.float32)[:, 0][None, :]),
        "wif": bf(np.asarray(W_inv_fert, np.float32)[0].reshape(c["DT"], 128).T),
        "WroT": bf(Wro_p.T).reshape(c["CTRO"], 128, H),
        "bro": np.ascontiguousarray(bro_p.reshape(c["PTRO"], 128).T),
        "WoT": bf(np.asarray(W_out, np.float32).T).reshape(c["PT2"], 128, V),
        "bout": np.ascontiguousarray(
            np.pad(np.asarray(b_out, np.float32), (0, c["VT"] * 128 - V))
            .reshape(c["VT"], 128).T),
        "embed": np.asarray(embed, np.float32),
    }
    return w


def prep_core_inputs(cfg, w, enc_core, labels_core, len_core):
    """Per-core input map (enc [Bl,T,D] f32, labels [Bl,N] int, len [Bl])."""
    c = _d(cfg)
    Bl, T, D, E, N = c["Bl"], c["T"], c["D"], c["E"], c["N"]
    enc = np.asarray(enc_core, np.float32)
    emb = w["embed"][np.asarray(labels_core).astype(np.int64)]        # [Bl,N,E]
    emb_sh = np.zeros_like(emb)
    emb_sh[:, 1:] = emb[:, :-1]
    embT = np.ascontiguousarray(emb_sh.transpose(2, 1, 0)).reshape(E, N * Bl)
    mask = np.where(np.arange(T)[None, :] < np.asarray(len_core)[:, None],
                    np.float32(0), np.float32(-1e4))
    m = {
        "encT": np.ascontiguousarray(enc.transpose(0, 2, 1)).astype(NPBF),
        "enc_t": np.ascontiguousarray(enc.reshape(Bl, 4, T // 4, D)).astype(NPBF),
        "embT": np.ascontiguousarray(embT).astype(NPBF).reshape(c["ET"], 128, c["NB"]),
        "maskadd": np.ascontiguousarray(mask, np.float32),
    }
    for k in ("WxT", "WcT", "WsT", "WencT", "benc", "bigo", "vatt", "wfb",
              "wif", "WroT", "bro", "WoT", "bout"):
        m[k] = w[k]
    return m


_built = None


def _get_built():
    global _built
    if _built is None:
        nc = bacc.Bacc("TRN2", target_bir_lowering=False, debug=False)
        build(nc, CFG)
        nc.compile()
        _built = nc
    return _built


def kernel(encoder_outputs, labels, enc_seq_len, embed, W_ih, b_ih, b_hh,
           W_s, W_enc_ctx, b_enc_ctx, v_att, W_inv_fert, W_fb,
           W_readout, b_readout, W_out, b_out):
    c = _d(CFG)
    Bl, N, V = c["Bl"], c["N"], c["V"]
    w = prep_weights(CFG, embed, W_ih, b_ih, b_hh, W_s, W_enc_ctx, b_enc_ctx,
                     v_att, W_inv_fert, W_fb, W_readout, b_readout, W_out, b_out)
    in_maps = []
    for i in range(NDEV):
        sl = slice(i * Bl, (i + 1) * Bl)
        in_maps.append(prep_core_inputs(
            CFG, w, encoder_outputs[sl], labels[sl], enc_seq_len[sl]))
    nc = _get_built()
    res = run_bass_kernel_spmd(nc, in_maps, core_ids=list(range(NDEV)))
    outs = []
    for i in range(NDEV):
        o = res.results[i]["out_d"]                      # [V, N*Bl]
        outs.append(o.reshape(V, N, Bl).transpose(2, 1, 0))
    return np.ascontiguousarray(np.concatenate(outs, axis=0), np.float32)


# revision 19
# speedup vs baseline: 1.3872x; 1.3872x over previous
"""AttentionLSTMDecoder — hand-written Bass/Tile kernel, 8-core data-parallel.

Sharding: batch B=16 -> 2 per NeuronCore (recurrence is batch-independent,
no collectives).  The whole 128-step recurrence + attention + output
projection runs as ONE Bass program per core.

Math restructuring (vs the nn.Module):
  * zero-state LSTM cell: forget gate is dead; only i/g/o rows of W_ih used.
  * c-zoneout chain is dead (h_new uses c_new) -> dropped.
  * x_t@W_ih part of gates hoisted into one PRE GEMM over all 128 steps.
  * sigmoid(x) = 0.5 + 0.5*tanh(x/2): the whole loop needs only {tanh, exp}
    which live in ONE ScalarE table set (no 2.7us table reloads per step).
  * per-step energy field X[a,t] = enc_ctx + s_t + wfb*accum built IN PSUM:
    rank-1 matmul (wfb x accum) + identity-matmul (enc_ctx) accumulate, then
    ONE ScalarE pass computes tanh(psum + s_bias) (s enters as the per-
    partition activation bias) -> VectorE untouched by the big tensor.
  * e = v . tanh(X) via matmul with v as 1-column stationary (a on partitions).
  * softmax without max-pass (|e| <= sum|v| ~ 26, exp stays in f32 range;
    masked lanes get -1e4 -> exp underflows to exact 0).  exp's free-dim
    accumulator gives the denominator for free.
  * maxout pairs are made partition-aligned by permuting W_readout rows
    (evens||odds) on the host, so MaxOut(2) is a plain tensor_tensor max.

Layouts (per core, B_loc=2):  a-major energies [a_part, t_free]; gates/h in
[h%128, (htile, b)]; s in [a%128, (atile, b)]; ctx in [d%128, (dtile, b)].
"""

import numpy as np
import ml_dtypes

import concourse.bass as bass
import concourse.bacc as bacc
import concourse.tile as tile
import concourse.mybir as mybir
from concourse.bass_utils import run_bass_kernel_spmd
from concourse.masks import make_identity

BF = mybir.dt.bfloat16
F32 = mybir.dt.float32
FP8 = mybir.dt.float8e4
NPBF = ml_dtypes.bfloat16
NPF8 = ml_dtypes.float8_e4m3
AF = mybir.ActivationFunctionType
ALU = mybir.AluOpType

NDEV = 8
ZH = 0.05

# full-size problem config
CFG = dict(Bl=2, T=500, D=512, A=1024, H=1024, E=640, N=128, V=10025)


def _d(cfg):
    Bl, T, D, A, H, E, N, V = (cfg[k] for k in ("Bl", "T", "D", "A", "H", "E", "N", "V"))
    G3 = 3 * H
    return dict(
        Bl=Bl, T=T, D=D, A=A, H=H, E=E, N=N, V=V, G3=G3,
        TQ=T // 4, DT=D // 128, AT=A // 128, HT=H // 128, ET=E // 128,
        GT=G3 // 128, RO=H + E + D, CTRO=(H + E + D) // 128, PTRO=H // 128,
        P2=H // 2, PT2=H // 256, NB=N * Bl, VT=(V + 127) // 128,
    )


def build(nc, cfg):
    c = _d(cfg)
    Bl, T, TQ, D, DT, A, AT, H, HT, E, ET, N, V, VT = (
        c[k] for k in ("Bl", "T", "TQ", "D", "DT", "A", "AT", "H", "HT", "E", "ET", "N", "V", "VT"))
    G3, GT, CTRO, PTRO, PT2, NB = (c[k] for k in ("G3", "GT", "CTRO", "PTRO", "PT2", "NB"))

    # ---------------- DRAM I/O ----------------
    def din(name, shape, dt=BF):
        return nc.dram_tensor(name, shape, dt, kind="ExternalInput")

    encT_d = din("encT", [Bl, D, T])
    enc_t_d = din("enc_t", [Bl, 4, TQ, D])
    embT_d = din("embT", [ET, 128, NB])
    WencT_d = din("WencT", [DT, 128, A])
    benc_d = din("benc", [128, AT], F32)
    wif_d = din("wif", [128, DT])
    WxT_d = din("WxT", [ET, 128, G3])
    bigo_d = din("bigo", [128, GT], F32)
    WcT_d = din("WcT", [DT, 128, G3], FP8)
    WsT_d = din("WsT", [HT, 128, A], FP8)
    wfb_d = din("wfb", [128, AT])
    vatt_d = din("vatt", [128, AT])
    mask_d = din("maskadd", [Bl, T], F32)
    WroT_d = din("WroT", [CTRO, 128, H])
    bro_d = din("bro", [128, PTRO], F32)
    WoT_d = din("WoT", [PT2, 128, V])
    bout_d = din("bout", [128, VT], F32)
    out_d = nc.dram_tensor("out_d", [V, NB], F32, kind="ExternalOutput")

    with tile.TileContext(nc) as tc:
        with (
            tc.tile_pool(name="const", bufs=1) as cp,
            tc.tile_pool(name="state", bufs=2) as sp,
            tc.tile_pool(name="bigstep", bufs=3) as bsp,
            tc.tile_pool(name="smallstep", bufs=3) as ssp,
            tc.tile_pool(name="rowstep", bufs=1) as rsp,
        ):
            # persistent SBUF tensors
            ident = cp.tile([128, 128], BF, tag="ident")
            make_identity(nc, ident[:])
            one32 = cp.tile([1, 1], F32, tag="one32")
            nc.vector.memset(one32[:], 1.0)

            encT = cp.tile([128, Bl, DT, T], BF, tag="encT")
            enc_t = cp.tile([128, Bl, 4, D], BF, tag="enc_t")
            embT = cp.tile([128, ET, NB], BF, tag="embT")
            WcT = cp.tile([128, DT, G3], FP8, tag="WcT")
            WsT = cp.tile([128, HT, A], FP8, tag="WsT")
            wfb = cp.tile([128, AT], BF, tag="wfb")
            ones128 = cp.tile([1, 128], BF, tag="ones128")
            nc.vector.memset(ones128[:], 1.0)
            vatt = cp.tile([128, AT], BF, tag="vatt")
            mask_b = [cp.tile([1, T], F32, tag=f"mask{b}", name=f"mask{b}") for b in range(Bl)]
            ivf_b = [cp.tile([1, T], F32, tag=f"ivf{b}", name=f"ivf{b}") for b in range(Bl)]
            enc_ctx = cp.tile([128, Bl, AT, T], BF, tag="enc_ctx")
            gx = cp.tile([128, GT, N, Bl], BF, tag="gx")
            h_seq = cp.tile([128, HT, N, Bl], BF, tag="h_seq")
            ctx_seq = cp.tile([128, DT, N, Bl], BF, tag="ctx_seq")

            for b in range(Bl):
                for dt in range(DT):
                    nc.sync.dma_start(encT[:, b, dt, :], encT_d[b, dt * 128:(dt + 1) * 128, :])
                for tt in range(4):
                    nc.sync.dma_start(enc_t[0:TQ, b, tt, :], enc_t_d[b, tt, :, :])
                nc.sync.dma_start(mask_b[b][:], mask_d[b:b + 1, :])
            for et in range(ET):
                nc.sync.dma_start(embT[:, et, :], embT_d[et])
            for dt in range(DT):
                nc.sync.dma_start(WcT[:, dt, :], WcT_d[dt])
            for ht in range(HT):
                nc.sync.dma_start(WsT[:, ht, :], WsT_d[ht])
            nc.sync.dma_start(wfb[:], wfb_d[:])
            nc.sync.dma_start(vatt[:], vatt_d[:])

            # ---------------- PRE phase ----------------
            with (
                tc.tile_pool(name="prew", bufs=1) as pw,
                tc.tile_pool(name="ps_ec", bufs=2, space="PSUM") as ps_ec,
                tc.tile_pool(name="ps_if", bufs=1, space="PSUM") as ps_if,
                tc.tile_pool(name="presb", bufs=1) as psb,
            ):
                WencT = pw.tile([128, DT, A], BF, tag="WencT")
                wif = pw.tile([128, DT], BF, tag="wif")
                benc = pw.tile([128, AT], F32, tag="benc")
                for dt in range(DT):
                    nc.sync.dma_start(WencT[:, dt, :], WencT_d[dt])
                nc.sync.dma_start(wif[:], wif_d[:])
                nc.sync.dma_start(benc[:], benc_d[:])

                # enc_ctx[b, at] = W_enc_ctx @ enc_b^T + b_enc_ctx   (a-major)
                for b in range(Bl):
                    for at in range(AT):
                        ecp = ps_ec.tile([128, T], F32, tag="ecp")
                        for dt in range(DT):
                            nc.tensor.matmul(
                                ecp[:], WencT[:, dt, at * 128:(at + 1) * 128],
                                encT[:, b, dt, :], start=(dt == 0), stop=(dt == DT - 1))
                        nc.scalar.activation(enc_ctx[:, b, at, :], ecp[:],
                                             AF.Identity, bias=benc[:, at:at + 1])

                # inv_fert_half = 0.25*tanh(x/2)+0.25, x = enc @ w_if
                for b in range(Bl):
                    ifp = ps_if.tile([1, T], F32, tag="ifp")
                    for dt in range(DT):
                        nc.tensor.matmul(ifp[:], wif[:, dt:dt + 1], encT[:, b, dt, :],
                                         start=(dt == 0), stop=(dt == DT - 1))
                    th0 = psb.tile([1, T], F32, tag="ifth")
                    nc.scalar.activation(th0[:], ifp[:], AF.Tanh, scale=0.5)
                    nc.vector.tensor_scalar(ivf_b[b][:], th0[:], 0.25, 0.25,
                                            op0=ALU.mult, op1=ALU.add)

            # gx[ht, n, b] = emb_shift @ Wx^T + b_igo
            with (
                tc.tile_pool(name="prew2", bufs=1) as pw2,
                tc.tile_pool(name="ps_gx", bufs=2, space="PSUM") as ps_gx,
            ):
                WxT = pw2.tile([128, ET, G3], BF, tag="WxT")
                bigo = pw2.tile([128, GT], F32, tag="bigo")
                for et in range(ET):
                    nc.sync.dma_start(WxT[:, et, :], WxT_d[et])
                nc.sync.dma_start(bigo[:], bigo_d[:])
                for ht in range(GT):
                    gxp = ps_gx.tile([128, NB], F32, tag="gxp")
                    for et in range(ET):
                        nc.tensor.matmul(gxp[:], WxT[:, et, ht * 128:(ht + 1) * 128],
                                         embT[:, et, :], start=(et == 0), stop=(et == ET - 1))
                    nc.scalar.activation(gx[:, ht, :, :], gxp[:],
                                         AF.Identity, bias=bigo[:, ht:ht + 1])

            # ---------------- recurrence ----------------
            with (
                tc.tile_pool(name="ps_g", bufs=1, space="PSUM") as ps_g,
                tc.tile_pool(name="ps_s", bufs=1, space="PSUM") as ps_s,
                tc.tile_pool(name="ps_x", bufs=2, space="PSUM") as ps_x,
                tc.tile_pool(name="ps_e", bufs=2, space="PSUM") as ps_e,
                tc.tile_pool(name="ps_t", bufs=1, space="PSUM") as ps_t,
                tc.tile_pool(name="ps_c", bufs=1, space="PSUM") as ps_c,
            ):
                # PE warm-up: ~5us of back-to-back matmuls so the HAM clock
                # gate un-throttles to 2.4 GHz right at the recurrence head.
                # Reading gx (the last PRE output) pins these after PRE in
                # the schedule so the burst fills the PRE->loop gap.
                for wu in range(22):
                    wup = ps_x.tile([128, T], F32, tag="xp", name="wup")
                    nc.tensor.matmul(wup[:, 0:NB], ident[:],
                                     gx[:, wu % GT, :, :], start=True, stop=True)

                h_prev = None
                ctx_sb = None
                accum = [None] * Bl
                accum_bf = [None] * Bl
                for t in range(N):
                    # ---- gates = gx[t] + ctx @ Wc^T ----
                    gates = ssp.tile([128, GT, Bl], BF, tag="gates")
                    if t == 0:
                        nc.vector.tensor_copy(gates[:], gx[:, :, 0, :])
                    else:
                        gp = ps_g.tile([128, GT, Bl], F32, tag="gp")
                        for ht in range(GT):
                            for dt in range(DT):
                                nc.tensor.matmul(
                                    gp[:, ht, :], WcT[:, dt, ht * 128:(ht + 1) * 128],
                                    ctx_sb[:, dt, :], start=(dt == 0), stop=(dt == DT - 1))
                        nc.vector.tensor_add(gates[:], gp[:], gx[:, :, t, :])

                    # ---- LSTM cell nonlinearities (tanh-only forms) ----
                    ti = ssp.tile([128, HT, Bl], BF, tag="ti")
                    tg = ssp.tile([128, HT, Bl], BF, tag="tg")
                    to = ssp.tile([128, HT, Bl], BF, tag="to")
                    nc.scalar.activation(ti[:], gates[:, 0:HT, :], AF.Tanh, scale=0.5)
                    nc.scalar.activation(tg[:], gates[:, HT:2 * HT, :], AF.Tanh)
                    nc.scalar.activation(to[:], gates[:, 2 * HT:3 * HT, :], AF.Tanh, scale=0.5)
                    u = ssp.tile([128, HT, Bl], BF, tag="u")
                    nc.vector.scalar_tensor_tensor(u[:], ti[:], 1.0, tg[:],
                                                   op0=ALU.add, op1=ALU.mult)
                    tc_ = ssp.tile([128, HT, Bl], BF, tag="tc_")
                    nc.scalar.activation(tc_[:], u[:], AF.Tanh, scale=0.5)
                    v2 = ssp.tile([128, HT, Bl], BF, tag="v2")
                    nc.vector.scalar_tensor_tensor(v2[:], to[:], 1.0, tc_[:],
                                                   op0=ALU.add, op1=ALU.mult)
                    h = sp.tile([128, HT, Bl], BF, tag="h")
                    if t == 0:
                        nc.vector.tensor_scalar_mul(h[:], v2[:], (1.0 - ZH) / 2.0)
                    else:
                        t1 = ssp.tile([128, HT, Bl], BF, tag="t1")
                        nc.vector.tensor_scalar_mul(t1[:], v2[:], (1.0 - ZH) / 2.0)
                        nc.vector.scalar_tensor_tensor(h[:], h_prev[:], ZH, t1[:],
                                                       op0=ALU.mult, op1=ALU.add)
                    nc.vector.tensor_copy(h_seq[:, :, t, :], h[:])
                    h_prev = h

                    # ---- s = h @ Ws^T  (a-major out) ----
                    sps = ps_s.tile([128, AT, Bl], F32, tag="sps")
                    for at in range(AT):
                        for ht in range(HT):
                            nc.tensor.matmul(
                                sps[:, at, :], WsT[:, ht, at * 128:(at + 1) * 128],
                                h[:, ht, :], start=(ht == 0), stop=(ht == HT - 1))
                    s_sb = ssp.tile([128, AT, Bl], F32, tag="s_sb")
                    nc.vector.tensor_copy(s_sb[:], sps[:])

                    # ---- energies + softmax per b ----
                    ewn = []
                    for b in range(Bl):
                        if t > 0:
                            # replicate accum across partitions via ones-matmul
                            arp = ps_x.tile([128, T], F32, tag="xp", name="arp")
                            nc.tensor.matmul(arp[:], ones128[:], accum_bf[b][:],
                                             start=True, stop=True)
                            arep = bsp.tile([128, T], BF, tag="arep")
                            nc.vector.tensor_copy(arep[:], arp[:])
                        eps = ps_e.tile([1, T], F32, tag="eps")
                        for at in range(AT):
                            th = bsp.tile([128, T], BF, tag="th")
                            if t == 0:
                                nc.scalar.activation(th[:], enc_ctx[:, b, at, :],
                                                     AF.Tanh, bias=s_sb[:, at, b:b + 1])
                            else:
                                xs = bsp.tile([128, T], BF, tag="xs")
                                nc.vector.scalar_tensor_tensor(
                                    xs[:], arep[:], wfb[:, at:at + 1],
                                    enc_ctx[:, b, at, :], op0=ALU.mult, op1=ALU.add)
                                nc.scalar.activation(th[:], xs[:], AF.Tanh,
                                                     bias=s_sb[:, at, b:b + 1])
                            nc.tensor.matmul(eps[:], vatt[:, at:at + 1], th[:],
                                             start=(at == 0), stop=(at == AT - 1))
                        em = rsp.tile([1, T], F32, tag=f"em{b}")
                        ew = rsp.tile([1, T], F32, tag=f"ew{b}")
                        ewn_t = rsp.tile([1, T], F32, tag=f"ewn{b}")
                        Ssum = ssp.tile([1, 1], F32, tag=f"Ssum{b}")
                        Sr = ssp.tile([1, 1], F32, tag=f"Sr{b}")
                        nc.vector.tensor_add(em[:], eps[:], mask_b[b][:])
                        nc.scalar.activation(ew[:], em[:], AF.Exp, accum_out=Ssum[:])
                        nc.vector.reciprocal(Sr[:], Ssum[:])
                        nc.vector.tensor_scalar_mul(ewn_t[:], ew[:], Sr[:])
                        ewn.append(ewn_t)

                        # accum += w * inv_fert/2
                        acc_new = sp.tile([1, T], F32, tag=f"accum{b}", name=f"accum{b}")
                        if t == 0:
                            nc.vector.tensor_mul(acc_new[:], ewn_t[:], ivf_b[b][:])
                        else:
                            wd = rsp.tile([1, T], F32, tag=f"wd{b}")
                            nc.vector.tensor_mul(wd[:], ewn_t[:], ivf_b[b][:])
                            nc.vector.tensor_add(acc_new[:], accum[b][:], wd[:])
                        accum[b] = acc_new
                        abf = sp.tile([1, T], BF, tag=f"accum_bf{b}", name=f"abf{b}", bufs=1)
                        nc.vector.tensor_copy(abf[:], acc_new[:])
                        accum_bf[b] = abf

                    # ---- ctx = w @ enc  (d-major out), via ew^T ----
                    ewT = ps_t.tile([128, 4, Bl], F32, tag="ewT")
                    for b in range(Bl):
                        for tt in range(4):
                            nc.tensor.matmul(ewT[0:TQ, tt, b:b + 1],
                                             ewn[b][0:1, tt * TQ:(tt + 1) * TQ],
                                             one32[:], start=True, stop=True)
                    ewT_sb = ssp.tile([128, 4, Bl], BF, tag="ewT_sb")
                    nc.vector.tensor_copy(ewT_sb[0:TQ, :, :], ewT[0:TQ, :, :])
                    cps = ps_c.tile([128, DT, Bl], F32, tag="cps")
                    for b in range(Bl):
                        for dt in range(DT):
                            for tt in range(4):
                                nc.tensor.matmul(
                                    cps[:, dt, b:b + 1],
                                    enc_t[0:TQ, b, tt, dt * 128:(dt + 1) * 128],
                                    ewT_sb[0:TQ, tt, b:b + 1],
                                    start=(tt == 0), stop=(tt == 3))
                    ctx_sb = sp.tile([128, DT, Bl], BF, tag="ctx_sb")
                    nc.vector.tensor_copy(ctx_sb[:], cps[:])
                    nc.scalar.copy(ctx_seq[:, :, t, :], cps[:])

            # ---------------- POST phase ----------------
            with (
                tc.tile_pool(name="postw", bufs=1) as pow_,
                tc.tile_pool(name="postwo", bufs=3) as powo,
                tc.tile_pool(name="post_sb", bufs=4) as post_sb,
            ):
                bro = pow_.tile([128, PTRO], F32, tag="bro")
                bout = pow_.tile([128, VT], F32, tag="bout")
                mo = pow_.tile([128, PT2, NB], BF, tag="mo")
                half = pow_.tile([128, NB], F32, tag="half")
                nc.sync.dma_start(bro[:], bro_d[:])
                nc.sync.dma_start(bout[:], bout_d[:])

                def ro_rhs(ct):
                    if ct < HT:
                        return h_seq[:, ct, :, :]
                    if ct < HT + ET:
                        return embT[:, ct - HT, :]
                    return ctx_seq[:, ct - HT - ET, :, :]

                # readout: all PTRO output tiles accumulate in one psum tensor,
                # each pt slice padded to a full bank (512 f32)
                with (
                    tc.tile_pool(name="postro", bufs=2) as powr,
                    tc.tile_pool(name="ps_rp", bufs=1, space="PSUM") as ps_rp,
                ):
                    rp = ps_rp.tile([128, PTRO, 512], F32, tag="rp")
                    for ct in range(CTRO):
                        WroTt = powr.tile([128, H], BF, tag="WroTt")
                        nc.sync.dma_start(WroTt[:], WroT_d[ct])
                        for pt in range(PTRO):
                            nc.tensor.matmul(rp[:, pt, 0:NB],
                                             WroTt[:, pt * 128:(pt + 1) * 128],
                                             ro_rhs(ct), start=(ct == 0),
                                             stop=(ct == CTRO - 1))
                    # maxout(2) with per-half biases, pairs partition-aligned
                    # by the host-side W_readout row permutation
                    for k in range(PT2):
                        nc.vector.tensor_scalar_add(half[:], rp[:, k + PT2, 0:NB],
                                                    bro[:, k + PT2:k + PT2 + 1])
                        nc.vector.scalar_tensor_tensor(mo[:, k, :], rp[:, k, 0:NB],
                                                       bro[:, k:k + 1], half[:],
                                                       op0=ALU.add, op1=ALU.max)

                with tc.tile_pool(name="ps_lp", bufs=3, space="PSUM") as ps_lp:
                    for vt in range(VT):
                        vp = min(128, V - vt * 128)
                        wo = powo.tile([128, PT2, 128], BF, tag="wo")
                        for ct in range(PT2):
                            nc.sync.dma_start(wo[:, ct, 0:vp],
                                              WoT_d[ct, :, vt * 128:vt * 128 + vp])
                        lp = ps_lp.tile([128, NB], F32, tag="lp")
                        for ct in range(PT2):
                            nc.tensor.matmul(lp[0:vp, :], wo[:, ct, 0:vp], mo[:, ct, :],
                                             start=(ct == 0), stop=(ct == PT2 - 1))
                        lg = post_sb.tile([128, NB], F32, tag="lg")
                        if vt % 2 == 0:
                            nc.vector.tensor_scalar_add(lg[0:vp, :], lp[0:vp, :],
                                                        bout[0:vp, vt:vt + 1])
                        else:
                            nc.scalar.activation(lg[0:vp, :], lp[0:vp, :], AF.Identity,
                                                 bias=bout[0:vp, vt:vt + 1])
                        nc.sync.dma_start(out_d[vt * 128:vt * 128 + vp, :], lg[0:vp, :])
    return nc


# ---------------------------------------------------------------------------
# host side
# ---------------------------------------------------------------------------

def prep_weights(cfg, embed, W_ih, b_ih, b_hh, W_s, W_enc_ctx, b_enc_ctx, v_att,
                 W_inv_fert, W_fb, W_readout, b_readout, W_out, b_out):
    """Static (input-independent) weight layout transforms -> shared arrays."""
    c = _d(cfg)
    H, E, A, D, V = c["H"], c["E"], c["A"], c["D"], c["V"]
    rows = np.r_[0:H, 2 * H:4 * H]
    Wih = np.asarray(W_ih, np.float32)
    Wx = Wih[rows, :E]
    Wc = Wih[rows, E:]
    b_igo = (np.asarray(b_ih, np.float32) + np.asarray(b_hh, np.float32))[rows]
    Wro = np.asarray(W_readout, np.float32)
    bro_f = np.asarray(b_readout, np.float32)
    Wro_p = np.concatenate([Wro[0::2], Wro[1::2]], axis=0)
    bro_p = np.concatenate([bro_f[0::2], bro_f[1::2]], axis=0)

    def bf(x):
        return np.ascontiguousarray(np.asarray(x, np.float32)).astype(NPBF)

    w = {
        "WxT": bf(Wx.T).reshape(c["ET"], 128, c["G3"]),
        "WcT": np.ascontiguousarray(Wc.T).astype(NPF8).reshape(c["DT"], 128, c["G3"]),
        "WsT": np.ascontiguousarray(np.asarray(W_s, np.float32).T).astype(NPF8).reshape(c["HT"], 128, A),
        "WencT": bf(np.asarray(W_enc_ctx, np.float32).T).reshape(c["DT"], 128, A),
        "benc": np.ascontiguousarray(
            np.asarray(b_enc_ctx, np.float32).reshape(c["AT"], 128).T),
        "bigo": np.ascontiguousarray(b_igo.reshape(c["GT"], 128).T),
        "vatt": bf(np.asarray(v_att, np.float32)[0].reshape(c["AT"], 128).T),
        "wfb": bf(np.asarray(W_fb, np.float32)[:, 0].reshape(c["AT"], 128).T),
        "wif": bf(np.asarray(W_inv_fert, np.float32)[0].reshape(c["DT"], 128).T),
        "WroT": bf(Wro_p.T).reshape(c["CTRO"], 128, H),
        "bro": np.ascontiguousarray(bro_p.reshape(c["PTRO"], 128).T),
        "WoT": bf(np.asarray(W_out, np.float32).T).reshape(c["PT2"], 128, V),
        "bout": np.ascontiguousarray(
            np.pad(np.asarray(b_out, np.float32), (0, c["VT"] * 128 - V))
            .reshape(c["VT"], 128).T),
        "embed": np.asarray(embed, np.float32),
    }
    return w


def prep_core_inputs(cfg, w, enc_core, labels_core, len_core):
    """Per-core input map (enc [Bl,T,D] f32, labels [Bl,N] int, len [Bl])."""
    c = _d(cfg)
    Bl, T, D, E, N = c["Bl"], c["T"], c["D"], c["E"], c["N"]
    enc = np.asarray(enc_core, np.float32)
    emb = w["embed"][np.asarray(labels_core).astype(np.int64)]        # [Bl,N,E]
    emb_sh = np.zeros_like(emb)
    emb_sh[:, 1:] = emb[:, :-1]
    embT = np.ascontiguousarray(emb_sh.transpose(2, 1, 0)).reshape(E, N * Bl)
    mask = np.where(np.arange(T)[None, :] < np.asarray(len_core)[:, None],
                    np.float32(0), np.float32(-1e4))
    m = {
        "encT": np.ascontiguousarray(enc.transpose(0, 2, 1)).astype(NPBF),
        "enc_t": np.ascontiguousarray(enc.reshape(Bl, 4, T // 4, D)).astype(NPBF),
        "embT": np.ascontiguousarray(embT).astype(NPBF).reshape(c["ET"], 128, c["NB"]),
        "maskadd": np.ascontiguousarray(mask, np.float32),
    }
    for k in ("WxT", "WcT", "WsT", "WencT", "benc", "bigo", "vatt", "wfb",
              "wif", "WroT", "bro", "WoT", "bout"):
        m[k] = w[k]
    return m


_built = None


def _get_built():
    global _built
    if _built is None:
        nc = bacc.Bacc("TRN2", target_bir_lowering=False, debug=False)
        build(nc, CFG)
        nc.compile()
        _built = nc
    return _built


def kernel(encoder_outputs, labels, enc_seq_len, embed, W_ih, b_ih, b_hh,
           W_s, W_enc_ctx, b_enc_ctx, v_att, W_inv_fert, W_fb,
           W_readout, b_readout, W_out, b_out):
    c = _d(CFG)
    Bl, N, V = c["Bl"], c["N"], c["V"]
    w = prep_weights(CFG, embed, W_ih, b_ih, b_hh, W_s, W_enc_ctx, b_enc_ctx,
                     v_att, W_inv_fert, W_fb, W_readout, b_readout, W_out, b_out)
    in_maps = []
    for i in range(NDEV):
        sl = slice(i * Bl, (i + 1) * Bl)
        in_maps.append(prep_core_inputs(
            CFG, w, encoder_outputs[sl], labels[sl], enc_seq_len[sl]))
    nc = _get_built()
    res = run_bass_kernel_spmd(nc, in_maps, core_ids=list(range(NDEV)))
    outs = []
    for i in range(NDEV):
        o = res.results[i]["out_d"]                      # [V, N*Bl]
        outs.append(o.reshape(V, N, Bl).transpose(2, 1, 0))
    return np.ascontiguousarray(np.concatenate(outs, axis=0), np.float32)
